# revision 50
# baseline (speedup 1.0000x reference)
"""Distributed Trainium2 Bass kernel for nn_AnyAttention (sparse attention).

Sharding (per the hint): 8 cores = 2 batches (data-parallel) x 4 head-groups
(tensor-parallel, 4 heads / 256 channels each). Attention never crosses head
shards; each core returns its bf16 partial row-parallel projection output
[C, Lq]; the host sums the 4 partials per batch in f32 and transposes.
b_proj rides on the hg==0 cores only.

Key structural choices (v2 — rebuilt around the timeline cost model):
 - Sparse attention: masked-out K columns (mask==1) are dropped on the host,
   padded to a common Lkp.  Pad columns come out of the LN-folded projection
   as exact zeros, so exp() gives a harmless 1.0 and a 0.0 entry in the v4a
   validity column excludes them from both the PV numerator and the softmax
   denominator.
 - FLIPPED LayerNorm stats: mu/msq per 128-token chunk are computed as
   matmul(lhsT=x_chunk[128C,128tok], rhs=ones[128,1]) chains accumulating
   into single PSUM columns — output free size 1, so the whole stats pass
   costs ~nothing on PE (vs ~20us for M=1 ones-row matmuls).  Stats come out
   as per-partition COLUMNS, which is exactly the layout the exp scale
   (k-side) and the v4a eviction scale (v-side) need.  Row forms (negmu for
   the rank-1 corrections, rstd_q for the q4 eviction) are built with cheap
   [128,1]->[1,128] PE transposes.
 - Three parallel DMA queues (transfers serialize per queue, run concurrently
   across queues): SP carries urows/wk/wv/k.h0/k.h1a/q.h1a/wp, ACT carries
   wq/q.h0/k.h1b/vbpack/q.h1b, and the Pool SWDGE queue carries the pos
   tensors as fused accum-adds onto x plus the v quarters.  x^2 squares run
   on DVE (bf16 2x) with the ACT engine picking up half of the pre-exp k
   squares (Square lives in the exp table set — no table swap).
 - Scores are computed transposed (S^T[k,q]) per (q-half, dt) group; the two
   heads of a kt share one 2-bank PSUM tile and ONE exp instruction whose
   per-partition scale is the k-chunk's SCALE*rstd column.  PV is computed
   transposed-back per (head, q-128-chunk) with out=[q, c+den]: the softmax
   denominator is a per-partition scalar (reciprocal read straight from
   PSUM), and normalized [q,c] tiles are PE-transposed in head pairs
   straight into o_sb's [c,q] layout.
 - The identity matrix for PE transposes is built on-device (memset +
   affine_select).  Tiny [1,1] warm matmuls anchored on early DMAs keep the
   PE p-state ramp alive through the load phase (<3us idle gaps).
 - Out-projection per q-half overlaps the second half of attention;
   evictions alternate ACT/DVE and stores alternate the SP/ACT queues to
   shorten the drain tail.
"""

import contextlib
import os
import numpy as np

import concourse.bass as bass
import concourse.tile as tile
from concourse import bacc, mybir
from concourse.bass_utils import run_bass_kernel_spmd

# The axon trace path imports antenv.axon_hooks; stub it if absent so a
# BASS_TRACE env var in the calling environment degrades gracefully.
try:
    import antenv.axon_hooks  # noqa: F401
except ImportError:
    import sys as _sys
    import types as _types
    _m = _types.ModuleType("antenv.axon_hooks")
    _m.get_axon_ntff_profile_hook = lambda: None
    _sys.modules["antenv.axon_hooks"] = _m

F32 = mybir.dt.float32
BF16 = mybir.dt.bfloat16

B = 2
LQ = 1024
LK = 2048
C = 1024
G = 16
HPC = 4          # heads per core
HC = 256         # head channels per core
CH = 64          # channels per head
SCALE = (C / G) ** -0.5   # 0.125
EPS = 1e-5
NCT = C // 128   # number of C tiles (8)
NDT = C // 128   # number of output-d tiles (8)
NCQ = LQ // 128  # q token chunks (8)

LAST_EXEC_NS = None
LAST_RESULTS = None
_NC_CACHE = {}


def _slices(total, step):
    out = []
    o = 0
    while o < total:
        s = min(step, total - o)
        out.append((o, s))
        o += s
    return out


def build_nc(Lkp, ln_identity=False, bproj_zero=False):
    NKT = Lkp // 128
    KH0 = (NKT // 2) * 128        # k token count in half 0
    KH1 = Lkp - KH0
    kh0c = KH0 // 128
    nc = bacc.Bacc(None, target_bir_lowering=False, debug=False)
    pspv_stack = contextlib.ExitStack()

    # ---- I/O (per-core shards) ----
    qT = nc.dram_tensor("qT", [C, LQ], BF16, kind="ExternalInput")
    kT = nc.dram_tensor("kT", [C, Lkp], BF16, kind="ExternalInput")
    vT = nc.dram_tensor("vT", [C, Lkp], BF16, kind="ExternalInput")
    wqT = nc.dram_tensor("wqT", [C, HC], BF16, kind="ExternalInput")
    wkT = nc.dram_tensor("wkT", [C, HC], BF16, kind="ExternalInput")
    wvT = nc.dram_tensor("wvT", [C, HC], BF16, kind="ExternalInput")
    wp = nc.dram_tensor("wp", [128, HPC // 2, C], BF16, kind="ExternalInput")
    bwqk_d = bwv_d = None
    if not ln_identity:
        bwqk_d = nc.dram_tensor("bwqk", [128, 4], F32, kind="ExternalInput")
        bwv_d = nc.dram_tensor("bwv", [128, HC], BF16, kind="ExternalInput")
    # vcol (validity) and bproj packed: [:, :NKT]=vcol, [:, NKT:]=bproj cols
    vbpack = nc.dram_tensor("vbpack", [128, NKT + NDT], F32, kind="ExternalInput")
    out = nc.dram_tensor("out", [C, LQ], BF16, kind="ExternalOutput")

    qT_r = qT.rearrange("(j p) t -> p j t", p=128)
    kT_r = kT.rearrange("(j p) t -> p j t", p=128)
    vT_r = vT.rearrange("(j p) t -> p j t", p=128)

    with tile.TileContext(nc) as tc:
        with (
            tc.tile_pool(name="persist", bufs=1) as P,
            tc.tile_pool(name="sq", bufs=3) as SQ,
            tc.tile_pool(name="psA", bufs=2, space="PSUM") as PSA,
            tc.tile_pool(name="psS", bufs=2, space="PSUM") as PSS,
        ):
            # ------------------------------------------------------------
            # on-device constants
            # ------------------------------------------------------------
            ones_col = P.tile([128, 1], BF16, tag="ones_col", name="ones_col")
            nc.vector.memset(ones_col, 1.0 / C)
            eps_t = P.tile([128, 1], F32, tag="eps_t", name="eps_t")
            nc.vector.memset(eps_t, EPS)
            ident_sb = P.tile([128, 128], BF16, tag="ident", name="ident")
            nc.vector.memset(ident_sb, 1.0)
            # iota(p, f) = p - f; keep 1.0 where p == f -> identity matrix
            nc.gpsimd.affine_select(ident_sb, ident_sb, pattern=[[-1, 128]],
                                    compare_op=mybir.AluOpType.is_equal,
                                    fill=0.0, base=0, channel_multiplier=1)
            ident32 = P.tile([128, 128], F32, tag="ident32", name="ident32")
            nc.vector.memset(ident32, 1.0)
            nc.gpsimd.affine_select(ident32, ident32, pattern=[[-1, 128]],
                                    compare_op=mybir.AluOpType.is_equal,
                                    fill=0.0, base=0, channel_multiplier=1)

            # ------------------------------------------------------------
            # tiles
            # ------------------------------------------------------------
            w_bf = {nm: P.tile([128, NCT, HC], BF16, tag=f"w_{nm}", name=f"w_{nm}")
                    for nm in ("q", "k", "v")}
            wp_bf = P.tile([128, HPC // 2, C], BF16, tag="wp", name="wp")
            vb_sb = P.tile([128, NKT + NDT], F32, tag="vb", name="vb")
            bwqk_sb = bwv_sb = None
            if not ln_identity:
                bwqk_sb = P.tile([128, 4], F32, tag="bwqk", name="bwqk")
                bwv_sb = P.tile([128, HC], BF16, tag="bwv", name="bwv")

            x_bf = {
                "q": P.tile([128, NCT, LQ], BF16, tag="x_q", name="x_q"),
                "k": P.tile([128, NCT, Lkp], BF16, tag="x_k", name="x_k"),
                "v": P.tile([128, NCT, Lkp], BF16, tag="x_v", name="x_v"),
            }
            ep_ctx = tc.tile_pool(name="epool", bufs=3)
            EP = ep_ctx.__enter__()
            sqp_ctx = tc.tile_pool(name="sqp", bufs=1)
            SQP = sqp_ctx.__enter__()
            sq_of = {
                "q": SQP.tile([128, NCT, LQ], BF16, tag="sq_q", name="sq_q"),
                "k": SQP.tile([128, NCT, Lkp], BF16, tag="sq_k", name="sq_k"),
                "v": SQP.tile([128, NCT, Lkp], BF16, tag="sq_v", name="sq_v"),
            }

            psst_ctx = tc.tile_pool(name="psST", bufs=1, space="PSUM")
            PSST = psst_ctx.__enter__()
            NCH = {"q": NCQ, "k": NKT, "v": NKT}
            CH0 = {"q": 0, "k": 2 * NCQ, "v": 2 * NCQ + 2 * NKT}
            stW = 2 * NCQ + 4 * NKT
            st_ps = PSST.tile([128, stW], F32, tag="st", name="st")
            rq_ps = PSST.tile([128, 512], F32, tag="rqp", name="rqp")

            def warm_on(ap2d):
                # p-state anchor: 1-column matmul into stats col 0 (the real
                # chain's start=True reset later overwrites the garbage)
                nc.tensor.matmul(st_ps[0:ap2d.shape[1], 0:1], ap2d,
                                 ones_col[0:ap2d.shape[0], :],
                                 start=True, stop=True)

            # ------------------------------------------------------------
            # DMA issue (per-queue order = priority order)
            # ------------------------------------------------------------
            warm_on(ones_col)  # t~0.4us anchor, no DMA dependency
            # SP queue (k.h0 first: its transfer is the critical-path pole)
            nc.sync.dma_start(out=x_bf["k"][:, 0:4, 0:KH0], in_=kT_r[:, 0:4, 0:KH0])
            warm_on(x_bf["k"][:, 0, 0:1])
            nc.sync.dma_start(out=x_bf["k"][:, 4:8, 0:KH0], in_=kT_r[:, 4:8, 0:KH0])
            warm_on(x_bf["k"][:, 4, 0:1])
            nc.sync.dma_start(out=w_bf["k"], in_=wkT.rearrange("(j p) d -> p j d", p=128))
            warm_on(w_bf["k"][:, 0, 0:1])

            # ACT queue (q.h0 first)
            nc.scalar.dma_start(out=x_bf["q"][:, 0:4, 0:512], in_=qT_r[:, 0:4, 0:512])
            warm_on(x_bf["q"][:, 0, 0:1])
            nc.scalar.dma_start(out=x_bf["q"][:, 4:8, 0:512], in_=qT_r[:, 4:8, 0:512])
            warm_on(x_bf["q"][:, 4, 0:1])
            nc.scalar.dma_start(out=w_bf["q"], in_=wqT.rearrange("(j p) d -> p j d", p=128))
            warm_on(w_bf["q"][:, 0, 0:1])
            if not ln_identity:
                nc.scalar.dma_start(out=bwqk_sb, in_=bwqk_d[:, :])
                nc.scalar.dma_start(out=bwv_sb, in_=bwv_d[:, :])

            # Pool queue: v in quarters
            vq = _slices(Lkp, 256)
            for (o, s) in vq:
                nc.gpsimd.dma_start(out=x_bf["v"][:, :, o:o + s], in_=vT_r[:, :, o:o + s])

            vcol_sb = vb_sb[:, 0:NKT]
            bproj_sb = vb_sb[:, NKT:NKT + NDT]

            # ------------------------------------------------------------
            # stats helpers
            # ------------------------------------------------------------
            def emit_mu_chains(nm, chunks):
                xb = x_bf[nm]
                for c in chunks:
                    for j in range(NCT):
                        nc.tensor.matmul(st_ps[:, CH0[nm] + c:CH0[nm] + c + 1],
                                         xb[:, j, c * 128:(c + 1) * 128], ones_col,
                                         start=(j == 0), stop=(j == NCT - 1))

            def emit_msq_chains(nm, chunks):
                sq = sq_of[nm]
                base = CH0[nm] + NCH[nm]
                for c in chunks:
                    for j in range(NCT):
                        nc.tensor.matmul(st_ps[:, base + c:base + c + 1],
                                         sq[:, j, c * 128:(c + 1) * 128], ones_col,
                                         start=(j == 0), stop=(j == NCT - 1))

            def emit_sq(nm, j0, jn, o, s, eng):
                sq, xb = sq_of[nm], x_bf[nm]
                if eng == "act":
                    nc.scalar.activation(sq[:, j0:j0 + jn, o:o + s],
                                         xb[:, j0:j0 + jn, o:o + s],
                                         mybir.ActivationFunctionType.Square)
                elif eng == "pool":
                    nc.gpsimd.tensor_mul(sq[:, j0:j0 + jn, o:o + s],
                                         xb[:, j0:j0 + jn, o:o + s],
                                         xb[:, j0:j0 + jn, o:o + s])
                else:
                    nc.vector.tensor_mul(sq[:, j0:j0 + jn, o:o + s],
                                         xb[:, j0:j0 + jn, o:o + s],
                                         xb[:, j0:j0 + jn, o:o + s])

            stc = {nm: P.tile([128, 2 * NCH[nm]], F32, tag=f"stc_{nm}", name=f"stc_{nm}")
                   for nm in ("q", "k", "v")}
            sdc = {nm: P.tile([128, NCH[nm]], F32, tag=f"sdc_{nm}", name=f"sdc_{nm}")
                   for nm in ("q", "k", "v")}
            rsc = {nm: P.tile([128, NCH[nm]], F32, tag=f"rsc_{nm}", name=f"rsc_{nm}")
                   for nm in ("q", "k", "v")}
            rk_cols = P.tile([128, NKT], F32, tag="rk_cols", name="rk_cols")
            rq_sb = P.tile([128, 512], BF16, tag="rq_sb", name="rq_sb")

            def emit_stat_math(nm, c0, cn, cp="dve"):
                """Evict psum stat cols for chunks [c0, c0+cn) and derive
                rstd = rsqrt(var + eps) by a fixed-seed Newton iteration."""
                cpy = nc.scalar.copy if cp == "act" else nc.vector.tensor_copy
                mu = stc[nm][:, c0:c0 + cn]
                msq = stc[nm][:, NCH[nm] + c0:NCH[nm] + c0 + cn]
                cpy(mu, st_ps[:, CH0[nm] + c0:CH0[nm] + c0 + cn])
                cpy(msq, st_ps[:, CH0[nm] + NCH[nm] + c0:
                        CH0[nm] + NCH[nm] + c0 + cn])
                mu2 = SQ.tile([128, cn], F32, tag="mu2", name="mu2", bufs=2)
                if cp == "act":
                    nc.scalar.square(mu2, mu)
                else:
                    nc.vector.tensor_mul(mu2, mu, mu)
                var = sdc[nm][:, c0:c0 + cn]
                nc.vector.tensor_sub(var, msq, mu2)
                ne = nc.gpsimd if cp == "pool_nr" else nc.vector
                h = SQ.tile([128, cn], F32, tag="nrh", name="nrh", bufs=2)
                ne.tensor_scalar(h, var, -0.5, -0.5 * EPS,
                                 mybir.AluOpType.mult, mybir.AluOpType.add)
                y = rsc[nm][:, c0:c0 + cn]
                # seed at rsqrt of the expected token variance: ~2 for q/k
                # (x + pos doubles the variance), ~1 for v
                ne.memset(y, 1.0 if nm == "v" else 0.7071068)
                a = SQ.tile([128, cn], F32, tag="nra", name="nra", bufs=2)
                for _nr in range(3 if nm == "v" else 2):
                    ne.tensor_mul(a, y, y)
                    ne.tensor_mul(a, h, a)
                    ne.scalar_tensor_tensor(y, a, 1.5, y,
                                            mybir.AluOpType.add,
                                            mybir.AluOpType.mult)

            def emit_rq_rows(c0, cn, cp="dve"):
                # rq_ps[:, c*128+i] = rstd_q[i, c] for every partition:
                # matmul with a stride-0 (column-replicated) bf16 stationary
                # operand against the bf16 identity transposes AND broadcasts
                # in one cheap PE instruction per chunk
                rqb = SQ.tile([128, cn], BF16, tag="rqb", name="rqb", bufs=2)
                nc.vector.tensor_copy(rqb, rsc["q"][:, c0:c0 + cn])
                for c in range(cn):
                    col = rqb[:, c:c + 1]
                    rep = bass.AP(tensor=col.tensor, offset=col.offset,
                                  ap=[list(col.ap[0]), [0, 128]])
                    nc.tensor.matmul(rq_ps[:, c * 128:c * 128 + 128],
                                     rep, ident_sb, start=True, stop=True)
                nc.vector.tensor_copy(rq_sb[:, 0:cn * 128], rq_ps[:, 0:cn * 128])

            # ------------------------------------------------------------
            # projections
            # ------------------------------------------------------------
            q4 = P.tile([128, 2, LQ], BF16, tag="q4", name="q4")
            k4 = P.tile([128, 2, Lkp], BF16, tag="k4", name="k4")

            def emit_qk4(nm, dt, o0, s0, ev="dve"):
                dest = q4 if nm == "q" else k4
                for (o, s) in [(o0 + oo, ss) for (oo, ss) in _slices(s0, 512)]:
                    ps = PSA.tile([128, 512], F32, tag="main", name="main")
                    for j in range(NCT):
                        nc.tensor.matmul(ps[:, :s], w_bf[nm][:, j, dt * 128:(dt + 1) * 128],
                                         x_bf[nm][:, j, o:o + s], start=(j == 0),
                                         stop=(j == NCT - 1))
                    if nm == "q":
                        nc.vector.tensor_mul(dest[:, dt, o:o + s], ps[:, :s],
                                             rq_sb[:, (o % 512):(o % 512) + s])
                        if not ln_identity:
                            nc.vector.tensor_scalar_add(
                                dest[:, dt, o:o + s], dest[:, dt, o:o + s],
                                bwqk_sb[:, dt:dt + 1])
                    elif ln_identity:
                        if ev == "act":
                            nc.scalar.copy(dest[:, dt, o:o + s], ps[:, :s])
                        else:
                            nc.vector.tensor_copy(dest[:, dt, o:o + s], ps[:, :s])
                    else:
                        # fold rstd_k at the eviction (exp scale becomes the
                        # plain softmax SCALE constant), then add beta@W
                        nc.vector.tensor_mul(dest[:, dt, o:o + s], ps[:, :s],
                                             rq_sb[:, (o % 512):(o % 512) + s])
                        nc.vector.tensor_scalar_add(
                            dest[:, dt, o:o + s], dest[:, dt, o:o + s],
                            bwqk_sb[:, 2 + dt:3 + dt])

            v4a = P.tile([128, NKT, HPC * (CH + 1)], BF16, tag="v4a", name="v4a")
            v4a_h = v4a.rearrange("p k (h x) -> p k h x", h=HPC)

            def emit_v4(kts):
                for kt in kts:
                    ps = PSA.tile([128, 512], F32, tag="main", name="main")
                    for j in range(NCT):
                        nc.tensor.matmul(ps[:, :HC], x_bf["v"][:, j, kt * 128:(kt + 1) * 128],
                                         w_bf["v"][:, j, :], start=(j == 0), stop=False)
                    nc.tensor.matmul(ps[:, :HC], negmu_row["v"][:, kt * 128:(kt + 1) * 128],
                                     u_row["v"][:, :], start=False,
                                     stop=(bw_row["v"] is None))
                    if bw_row["v"] is not None:
                        nc.tensor.matmul(ps[:, :HC], sdrow["v"][:, kt * 128:(kt + 1) * 128],
                                         bw_row["v"][:, :], start=False, stop=True)
                    nc.vector.tensor_scalar_mul(
                        v4a_h[:, kt, :, 0:CH],
                        ps[:, 0:HC].rearrange("p (h x) -> p h x", h=HPC),
                        rsc["v"][:, kt:kt + 1])

            # ------------------------------------------------------------
            # attention pipeline
            # ------------------------------------------------------------
            groups = [(0, 512, 0), (0, 512, 1), (512, 512, 1), (512, 512, 0)]
            et_of = {}

            def emit_sexp(g, kts=None, alloc=True):
                (o, s, dt) = groups[g]
                if alloc:
                    et_of[g] = EP.tile([128, NKT, 2, 512], BF16, tag="E", name="E")
                et = et_of[g]
                for kt in (range(NKT) if kts is None else kts):
                    ps = PSS.tile([128, 2, 512], F32, tag="S", name="S")
                    for hh in range(2):
                        pb = 64 * hh
                        nc.tensor.matmul(ps[:, hh, :s],
                                         k4[pb:pb + CH, dt, kt * 128:(kt + 1) * 128],
                                         q4[pb:pb + CH, dt, o:o + s],
                                         start=True, stop=True)
                    nc.scalar.activation(et[:, kt, :, :s], ps[:, :, :s],
                                         mybir.ActivationFunctionType.Exp,
                                         scale=rk_cols[:, kt:kt + 1])

            o_sb = P.tile([128, HPC // 2, LQ], BF16, tag="o_sb", name="o_sb")
            ot_of = {}
            PSPV_box = [None]

            def emit_pv_unit(g, hh, qc):
                (o, s, dt) = groups[g]
                PSPV = PSPV_box[0]
                et = et_of[g]
                h = 2 * dt + hh
                qo = o + qc * 128
                ppv = PSPV.tile([128, CH + 1], F32, tag="pv", name="pv")
                for kt in range(NKT):
                    nc.tensor.matmul(ppv[:, :],
                                     et[:, kt, hh, qc * 128:(qc + 1) * 128],
                                     v4a[:, kt, h * (CH + 1):(h + 1) * (CH + 1)],
                                     start=(kt == 0), stop=(kt == NKT - 1))
                rdc = SQ.tile([128, 1], F32, tag="rdc", name="rdc", bufs=3)
                nc.vector.reciprocal_approx_fast(out=rdc, in_=ppv[:, CH:CH + 1])
                o_t = SQ.tile([128, CH], BF16, tag="o_t", name="o_t", bufs=17)
                nc.vector.tensor_scalar_mul(o_t, ppv[:, 0:CH], rdc)
                ot_of[(h, qo)] = o_t

            def emit_T_unit(o, qc):
                # transpose head pairs {t, t+2} of q-chunk qc into o_sb
                PSPV = PSPV_box[0]
                qo = o + qc * 128
                for t in range(2):
                    pst = PSPV.tile([128, 128], BF16, tag="pv", name="pv")
                    nc.tensor.transpose(pst[0:CH, :], ot_of.pop((t, qo)),
                                        ident_sb, tile_position=(0, 0))
                    nc.tensor.transpose(pst[CH:128, :], ot_of.pop((t + 2, qo)),
                                        ident_sb, tile_position=(0, CH))
                    nc.vector.tensor_copy(o_sb[:, t, qo:qo + 128], pst)

            def emit_proj_unit(dt2, o, s, evict, store):
                ps = PSA.tile([128, 512], F32, tag="main", name="main")
                for t in range(HPC // 2):
                    nc.tensor.matmul(ps[:, :s], wp_bf[:, t, dt2 * 128:(dt2 + 1) * 128],
                                     o_sb[:, t, o:o + s], start=(t == 0),
                                     stop=(t == HPC // 2 - 1))
                ot = SQ.tile([128, 512], BF16, tag="ot", name="ot", bufs=4)
                if not bproj_zero:
                    nc.vector.tensor_scalar_add(ot[:, :s], ps[:, :s],
                                                bproj_sb[:, dt2:dt2 + 1])
                elif evict == "act":
                    nc.scalar.activation(ot[:, :s], ps[:, :s],
                                         mybir.ActivationFunctionType.Copy)
                else:
                    nc.vector.tensor_copy(ot[:, :s], ps[:, :s])
                store.dma_start(out=out[dt2 * 128:(dt2 + 1) * 128, o:o + s],
                                in_=ot[:, :s])

            # ------------------------------------------------------------
            # ordered emission: pre-exp phase, then one unified PE stream
            # where score units (2 matmuls + 1 exp) interleave with filler
            # units (projections, v4, PV, transposes, out-proj) so the PE
            # queue never parks behind the exp-paced score tiles.
            # ------------------------------------------------------------
            emit_mu_chains("v", range(NKT))

            # q.h0: full squares + stat math on DVE (free early; shortest
            # latency to the first projections/scores)
            emit_sq("q", 0, 4, 0, 512, "dve")
            emit_sq("q", 4, 4, 0, 512, "dve")
            emit_mu_chains("q", range(4))
            emit_msq_chains("q", range(4))
            emit_stat_math("q", 0, 4)
            emit_rq_rows(0, 4)

            # k.h0: squares fully on ACT (its only pre-exp job); math on DVE
            emit_sq("k", 4, 4, 0, KH0, "act")
            emit_sq("k", 0, 4, 0, KH0, "act")
            emit_mu_chains("k", range(kh0c))
            emit_msq_chains("k", range(kh0c))
            emit_stat_math("k", 0, kh0c)
            nc.vector.tensor_scalar_mul(rk_cols[:, 0:kh0c], rsc["k"][:, 0:kh0c], SCALE)

            emit_qk4("q", 0, 0, 512)
            emit_qk4("k", 0, 0, KH0)
            emit_qk4("k", 0, KH0, KH1)

            # deferred second-half loads (gens issue behind the urgent ones)
            nc.sync.dma_start(out=x_bf["k"][:, 0:4, KH0:Lkp], in_=kT_r[:, 0:4, KH0:Lkp])
            nc.scalar.dma_start(out=x_bf["k"][:, 4:8, KH0:Lkp], in_=kT_r[:, 4:8, KH0:Lkp])
            nc.sync.dma_start(out=x_bf["q"][:, 0:4, 512:1024], in_=qT_r[:, 0:4, 512:1024])
            nc.scalar.dma_start(out=x_bf["q"][:, 4:8, 512:1024], in_=qT_r[:, 4:8, 512:1024])

            # k.h1 / v / q.h1 squares on Pool (non-critical latency)
            emit_sq("v", 0, NCT, vq[0][0], vq[0][1], "pool")
            emit_sq("v", 0, NCT, vq[1][0], vq[1][1], "pool")
            emit_sq("k", 0, 4, KH0, KH1, "pool")
            emit_sq("k", 4, 4, KH0, KH1, "pool")
            emit_sq("q", 4, 4, 512, 512, "pool")
            emit_sq("q", 0, 4, 512, 512, "pool")
            emit_sq("v", 0, NCT, vq[2][0], vq[2][1], "pool")
            emit_sq("v", 0, NCT, vq[3][0], vq[3][1], "pool")
            # deferred (non-urgent) loads: issue now, behind the urgent gens
            nc.sync.dma_start(out=w_bf["v"], in_=wvT.rearrange("(j p) d -> p j d", p=128))
            nc.scalar.dma_start(out=vb_sb, in_=vbpack[:, :])
            nc.sync.dma_start(out=wp_bf, in_=wp[:, :, :])

            # ---- filler units ----
            def f_kh1_stats():
                emit_mu_chains("k", range(kh0c, NKT))
                emit_msq_chains("k", range(kh0c, NKT))
                emit_stat_math("k", kh0c, NKT - kh0c)
                nc.vector.tensor_scalar_mul(rk_cols[:, kh0c:NKT],
                                            rsc["k"][:, kh0c:NKT], SCALE)

            def f_v_stats():
                emit_msq_chains("v", range(NKT))
                emit_stat_math("v", 0, NKT)
                for h in range(HPC):
                    nc.vector.tensor_copy(v4a_h[:, :, h, CH:CH + 1], vcol_sb[:, :, None])

            def f_qh1_stats():
                emit_mu_chains("q", range(4, NCQ))
                emit_msq_chains("q", range(4, NCQ))

            def f_qh1_math():
                emit_stat_math("q", 4, NCQ - 4)
                emit_rq_rows(4, NCQ - 4)

            def f_pools():
                sqp_ctx.__exit__(None, None, None)
                psst_ctx.__exit__(None, None, None)
                PSPV_box[0] = pspv_stack.enter_context(
                    tc.tile_pool(name="psPV", bufs=2, space="PSUM"))

            fillers = [
                (0.3, f_kh1_stats),
                (2.0, lambda: emit_qk4("q", 1, 0, 512)),
                (2.0, lambda: emit_qk4("k", 1, 0, KH0)),
                (2.0, lambda: emit_qk4("k", 1, KH0, KH1)),
                (0.1, f_qh1_stats),
                (0.4, f_qh1_math),
                (2.0, lambda: emit_qk4("q", 1, 512, 512)),
                (2.0, lambda: emit_qk4("q", 0, 512, 512)),
                (0.3, f_v_stats),
            ]
            for kt0 in range(NKT):
                fillers.append((1.0, lambda kt0=kt0: emit_v4([kt0])))
            fillers += [
                (0.0, f_pools),
            ]
            for hh in range(2):
                for qc in range(4):
                    fillers.append((0.25, lambda hh=hh, qc=qc: emit_pv_unit(0, hh, qc)))
            for qc in range(4):
                fillers.append((0.25, lambda qc=qc: emit_pv_unit(1, 0, qc)))
                fillers.append((0.25, lambda qc=qc: emit_pv_unit(1, 1, qc)))
                fillers.append((0.15, lambda qc=qc: emit_T_unit(0, qc)))
            for dt2 in range(NDT):
                fillers.append((0.5, lambda dt2=dt2: emit_proj_unit(dt2, 0, 512,
                                                                   "dve", nc.sync)))
            for hh in range(2):
                for qc in range(4):
                    fillers.append((0.25, lambda hh=hh, qc=qc: emit_pv_unit(2, hh, qc)))

            # ---- unified S stream: budget-paced filler pumping ----
            fi = [0]
            debt = [0.0]

            def pump(budget):
                debt[0] += budget
                while fi[0] < len(fillers) and debt[0] >= fillers[fi[0]][0]:
                    cost, fn = fillers[fi[0]]
                    fn()
                    debt[0] -= cost
                    fi[0] += 1

            def pump_through(idx):
                # force-emit fillers [0, idx) regardless of budget (ordering
                # requirement: later S units read tiles these fillers write)
                while fi[0] < idx:
                    fillers[fi[0]][1]()
                    fi[0] += 1

            # filler indices that must precede each score group / kt
            import os as _os
            _budget = float(_os.environ.get("PUMP_BUDGET", "1.0"))
            # g3's exp reuses E(g0)'s buffer: all 8 PV(g0) units (and the v4
            # chain they depend on) must drain before S(g3) starts
            need_g = {1: 4, 2: 8, 3: 9 + NKT}
            need_kh1 = 1
            for g in range(4):
                if g in need_g:
                    pump_through(need_g[g])
                emit_sexp(g, kts=[], alloc=True)
                for kt in range(NKT):
                    if g == 0 and kt == kh0c:
                        pump_through(need_kh1)
                    emit_sexp(g, kts=[kt], alloc=False)
                    # ~0.61us of exp-paced slack per S slot, minus overheads
                    pump(_budget)
            # flush remaining fillers behind the S stream
            debt[0] = 1e9
            pump(0.0)

            # ---- tail: PV(g3) per q-chunk, transposes, out-proj 256-wide ----
            for qc in range(4):
                emit_pv_unit(3, 0, qc)
                emit_pv_unit(3, 1, qc)
                emit_T_unit(512, qc)
                if qc == 1:
                    for dt2 in range(NDT):
                        emit_proj_unit(dt2, 512, 256,
                                       "act" if dt2 % 2 else "dve",
                                       nc.scalar if dt2 % 2 else nc.sync)
            for dt2 in range(NDT):
                emit_proj_unit(dt2, 768, 256,
                               "act" if dt2 % 2 else "dve",
                               nc.scalar if dt2 % 2 else nc.sync)

            pspv_stack.close()
            ep_ctx.__exit__(None, None, None)

    nc.compile()
    return nc


def prepare_in_maps(q, k, v, qpos, kpos, mask,
                    ln_q_w, ln_q_b, ln_k_w, ln_k_b, ln_v_w, ln_v_b,
                    w_q, w_k, w_v, w_proj, b_proj):
    import ml_dtypes
    bf = ml_dtypes.bfloat16
    f = np.float32
    q = np.asarray(q, f); k = np.asarray(k, f); v = np.asarray(v, f)
    qpos = np.asarray(qpos, f).reshape(B, LQ, C)
    kpos = np.asarray(kpos, f).reshape(B, LK, C)
    mask = np.asarray(mask)

    keeps = [np.flatnonzero(mask[b, 0, 0] == 0) for b in range(B)]
    Lkp = max(256, -(-max(len(kp) for kp in keeps) // 128) * 128)
    NKT = Lkp // 128

    def colmajor(vec, ntiles):
        return np.ascontiguousarray(vec.reshape(ntiles, 128).T.astype(f))

    gammas = {"q": np.asarray(ln_q_w, f), "k": np.asarray(ln_k_w, f),
              "v": np.asarray(ln_v_w, f)}
    betas = {"q": np.asarray(ln_q_b, f), "k": np.asarray(ln_k_b, f),
             "v": np.asarray(ln_v_b, f)}
    Ws = {"q": np.asarray(w_q, f), "k": np.asarray(w_k, f), "v": np.asarray(w_v, f)}
    ident = all(np.all(gammas[n] == 1.0) for n in "qkv") \
        and all(np.all(betas[n] == 0.0) for n in "qkv")

    in_maps = []
    for core in range(8):
        b, hg = core // 4, core % 4
        kp = keeps[b]
        nk = len(kp)
        hs = slice(hg * HC, (hg + 1) * HC)

        def padT(x2d):  # [n, C] -> [C, Lkp] bf16
            outp = np.zeros((C, Lkp), bf)
            outp[:, :x2d.shape[0]] = x2d.T.astype(bf)
            return np.ascontiguousarray(outp)

        # W'' = gamma-scaled W with the LayerNorm mean-centering folded in:
        # W''[c,d] = gamma[c]W[d,c] - u[d]/C with u = colsum(gamma*W), so
        # W''^T x = W'x - mu(x)*u exactly; bW = beta @ W
        wT_eff = {}
        bw_rows = np.zeros((3, HC), f)
        for i, nm in enumerate("qkv"):
            wt = (Ws[nm][hs, :] * gammas[nm][None, :]).T.astype(bf).astype(f)
            u = wt.sum(0)
            wT_eff[nm] = np.ascontiguousarray((wt - u[None, :] / C).astype(bf))
            bw_rows[i] = betas[nm].astype(f) @ Ws[nm][hs, :].T.astype(bf).astype(f)

        vcol_np = np.zeros(Lkp, f)
        vcol_np[:nk] = 1.0
        vb = np.concatenate(
            [colmajor(vcol_np, NKT),
             colmajor(np.asarray(b_proj, f) if hg == 0 else np.zeros(C, f), NDT)],
            axis=1)
        m = {
            "qT": np.ascontiguousarray((q[b] + qpos[b]).T.astype(bf)),
            "kT": padT((k[b] + kpos[b])[kp]),
            "vT": padT(v[b][kp]),
            "wqT": wT_eff["q"],
            "wkT": wT_eff["k"],
            "wvT": wT_eff["v"],
            # wp[64*(h//2)+p, h%2, d] = w_proj[d, hg*256 + 64h + p]:
            # heads 0/1 on the lower partition halves of slots 0/1, heads
            # 2/3 on the upper halves (matches the o_sb placement)
            "wp": np.ascontiguousarray(
                np.asarray(w_proj, f)[:, hs].T.reshape(2, HPC // 2, CH, C)
                .transpose(0, 2, 1, 3).reshape(128, HPC // 2, C).astype(bf)),
            "vbpack": np.ascontiguousarray(vb),
        }
        if not ident:
            # bias terms: per-partition columns for q/k (d on partitions, 2
            # dt tiles) and a partition-replicated tile for v (d on free dim)
            m["bwqk"] = np.ascontiguousarray(np.stack(
                [bw_rows[0].reshape(2, 128).T, bw_rows[1].reshape(2, 128).T],
                axis=1).reshape(128, 4).astype(f))
            m["bwv"] = np.ascontiguousarray(
                np.broadcast_to(bw_rows[2][None, :], (128, HC)).astype(bf))
        in_maps.append(m)
    return in_maps, Lkp, ident


def kernel(**inputs):
    global LAST_EXEC_NS, LAST_RESULTS
    f = np.float32
    in_maps, Lkp, ident = prepare_in_maps(**inputs)
    bz = bool(np.all(np.asarray(inputs["b_proj"]) == 0.0))
    key = (Lkp, ident, bz)
    nc = _NC_CACHE.get(key)
    if nc is None:
        nc = build_nc(Lkp, ln_identity=ident, bproj_zero=bz)
        _NC_CACHE[key] = nc
    trace = os.environ.get("KERNEL_TRACE", "0") == "1"
    res = run_bass_kernel_spmd(nc, in_maps, core_ids=list(range(8)), trace=trace)
    LAST_EXEC_NS = res.exec_time_ns
    LAST_RESULTS = res

    out_full = np.zeros((B, LQ, C), f)
    for b in range(B):
        acc = np.zeros((C, LQ), f)
        for hg in range(4):
            acc += res.results[b * 4 + hg]["out"].astype(f)
        out_full[b] = acc.T
    return out_full


# revision 51
# speedup vs baseline: 1.0376x; 1.0376x over previous
"""Distributed Trainium2 Bass kernel for nn_AnyAttention (sparse attention).

Sharding (per the hint): 8 cores = 2 batches (data-parallel) x 4 head-groups
(tensor-parallel, 4 heads / 256 channels each). Attention never crosses head
shards; each core returns its bf16 partial row-parallel projection output
[C, Lq]; the host sums the 4 partials per batch in f32 and transposes.
b_proj rides on the hg==0 cores only.

Key structural choices (v2 — rebuilt around the timeline cost model):
 - Sparse attention: masked-out K columns (mask==1) are dropped on the host,
   padded to a common Lkp.  Pad columns come out of the LN-folded projection
   as exact zeros, so exp() gives a harmless 1.0 and a 0.0 entry in the v4a
   validity column excludes them from both the PV numerator and the softmax
   denominator.
 - FLIPPED LayerNorm stats: mu/msq per 128-token chunk are computed as
   matmul(lhsT=x_chunk[128C,128tok], rhs=ones[128,1]) chains accumulating
   into single PSUM columns — output free size 1, so the whole stats pass
   costs ~nothing on PE (vs ~20us for M=1 ones-row matmuls).  Stats come out
   as per-partition COLUMNS, which is exactly the layout the exp scale
   (k-side) and the v4a eviction scale (v-side) need.  Row forms (negmu for
   the rank-1 corrections, rstd_q for the q4 eviction) are built with cheap
   [128,1]->[1,128] PE transposes.
 - Three parallel DMA queues (transfers serialize per queue, run concurrently
   across queues): SP carries urows/wk/wv/k.h0/k.h1a/q.h1a/wp, ACT carries
   wq/q.h0/k.h1b/vbpack/q.h1b, and the Pool SWDGE queue carries the pos
   tensors as fused accum-adds onto x plus the v quarters.  x^2 squares run
   on DVE (bf16 2x) with the ACT engine picking up half of the pre-exp k
   squares (Square lives in the exp table set — no table swap).
 - Scores are computed transposed (S^T[k,q]) per (q-half, dt) group; the two
   heads of a kt share one 2-bank PSUM tile and ONE exp instruction whose
   per-partition scale is the k-chunk's SCALE*rstd column.  PV is computed
   transposed-back per (head, q-128-chunk) with out=[q, c+den]: the softmax
   denominator is a per-partition scalar (reciprocal read straight from
   PSUM), and normalized [q,c] tiles are PE-transposed in head pairs
   straight into o_sb's [c,q] layout.
 - The identity matrix for PE transposes is built on-device (memset +
   affine_select).  Tiny [1,1] warm matmuls anchored on early DMAs keep the
   PE p-state ramp alive through the load phase (<3us idle gaps).
 - Out-projection per q-half overlaps the second half of attention;
   evictions alternate ACT/DVE and stores alternate the SP/ACT queues to
   shorten the drain tail.
"""

import contextlib
import os
import numpy as np

import concourse.bass as bass
import concourse.tile as tile
from concourse import bacc, mybir
from concourse.bass_utils import run_bass_kernel_spmd

# The axon trace path imports antenv.axon_hooks; stub it if absent so a
# BASS_TRACE env var in the calling environment degrades gracefully.
try:
    import antenv.axon_hooks  # noqa: F401
except ImportError:
    import sys as _sys
    import types as _types
    _m = _types.ModuleType("antenv.axon_hooks")
    _m.get_axon_ntff_profile_hook = lambda: None
    _sys.modules["antenv.axon_hooks"] = _m

F32 = mybir.dt.float32
BF16 = mybir.dt.bfloat16

B = 2
LQ = 1024
LK = 2048
C = 1024
G = 16
HPC = 4          # heads per core
HC = 256         # head channels per core
CH = 64          # channels per head
SCALE = (C / G) ** -0.5   # 0.125
EPS = 1e-5
NCT = C // 128   # number of C tiles (8)
NDT = C // 128   # number of output-d tiles (8)
NCQ = LQ // 128  # q token chunks (8)

LAST_EXEC_NS = None
LAST_RESULTS = None
_NC_CACHE = {}


def _slices(total, step):
    out = []
    o = 0
    while o < total:
        s = min(step, total - o)
        out.append((o, s))
        o += s
    return out


def build_nc(Lkp, ln_identity=False, bproj_zero=False):
    NKT = Lkp // 128
    KH0 = (NKT // 2) * 128        # k token count in half 0
    KH1 = Lkp - KH0
    kh0c = KH0 // 128
    nc = bacc.Bacc(None, target_bir_lowering=False, debug=False)
    pspv_stack = contextlib.ExitStack()

    # ---- I/O (per-core shards) ----
    qT = nc.dram_tensor("qT", [C, LQ], BF16, kind="ExternalInput")
    kT = nc.dram_tensor("kT", [C, Lkp], BF16, kind="ExternalInput")
    vT = nc.dram_tensor("vT", [C, Lkp], BF16, kind="ExternalInput")
    wqT = nc.dram_tensor("wqT", [C, HC], BF16, kind="ExternalInput")
    wkT = nc.dram_tensor("wkT", [C, HC], BF16, kind="ExternalInput")
    wvT = nc.dram_tensor("wvT", [C, HC], BF16, kind="ExternalInput")
    wp = nc.dram_tensor("wp", [128, HPC // 2, C], BF16, kind="ExternalInput")
    bwqk_d = bwv_d = None
    if not ln_identity:
        bwqk_d = nc.dram_tensor("bwqk", [128, 4], F32, kind="ExternalInput")
        bwv_d = nc.dram_tensor("bwv", [128, HC], BF16, kind="ExternalInput")
    # vcol (validity) and bproj packed: [:, :NKT]=vcol, [:, NKT:]=bproj cols
    vbpack = nc.dram_tensor("vbpack", [128, NKT + NDT], F32, kind="ExternalInput")
    out = nc.dram_tensor("out", [C, LQ], BF16, kind="ExternalOutput")

    qT_r = qT.rearrange("(j p) t -> p j t", p=128)
    kT_r = kT.rearrange("(j p) t -> p j t", p=128)
    vT_r = vT.rearrange("(j p) t -> p j t", p=128)

    with tile.TileContext(nc) as tc:
        with (
            tc.tile_pool(name="persist", bufs=1) as P,
            tc.tile_pool(name="sq", bufs=3) as SQ,
            tc.tile_pool(name="psA", bufs=2, space="PSUM") as PSA,
            tc.tile_pool(name="psS", bufs=2, space="PSUM") as PSS,
        ):
            # ------------------------------------------------------------
            # on-device constants
            # ------------------------------------------------------------
            ones_col = P.tile([128, 1], BF16, tag="ones_col", name="ones_col")
            nc.vector.memset(ones_col, 1.0 / C)
            eps_t = P.tile([128, 1], F32, tag="eps_t", name="eps_t")
            nc.vector.memset(eps_t, EPS)
            ident_sb = P.tile([128, 128], BF16, tag="ident", name="ident")
            nc.vector.memset(ident_sb, 1.0)
            # iota(p, f) = p - f; keep 1.0 where p == f -> identity matrix
            nc.gpsimd.affine_select(ident_sb, ident_sb, pattern=[[-1, 128]],
                                    compare_op=mybir.AluOpType.is_equal,
                                    fill=0.0, base=0, channel_multiplier=1)
            ident32 = P.tile([128, 128], F32, tag="ident32", name="ident32")
            nc.vector.memset(ident32, 1.0)
            nc.gpsimd.affine_select(ident32, ident32, pattern=[[-1, 128]],
                                    compare_op=mybir.AluOpType.is_equal,
                                    fill=0.0, base=0, channel_multiplier=1)

            # ------------------------------------------------------------
            # tiles
            # ------------------------------------------------------------
            w_bf = {nm: P.tile([128, NCT, HC], BF16, tag=f"w_{nm}", name=f"w_{nm}")
                    for nm in ("q", "k", "v")}
            wp_bf = P.tile([128, HPC // 2, C], BF16, tag="wp", name="wp")
            vb_sb = P.tile([128, NKT + NDT], F32, tag="vb", name="vb")
            bwqk_sb = bwv_sb = None
            if not ln_identity:
                bwqk_sb = P.tile([128, 4], F32, tag="bwqk", name="bwqk")
                bwv_sb = P.tile([128, HC], BF16, tag="bwv", name="bwv")

            x_bf = {
                "q": P.tile([128, NCT, LQ], BF16, tag="x_q", name="x_q"),
                "k": P.tile([128, NCT, Lkp], BF16, tag="x_k", name="x_k"),
                "v": P.tile([128, NCT, Lkp], BF16, tag="x_v", name="x_v"),
            }
            ep_ctx = tc.tile_pool(name="epool", bufs=3)
            EP = ep_ctx.__enter__()
            sqp_ctx = tc.tile_pool(name="sqp", bufs=1)
            SQP = sqp_ctx.__enter__()
            sq_of = {
                "q": SQP.tile([128, NCT, LQ], BF16, tag="sq_q", name="sq_q"),
                "k": SQP.tile([128, NCT, Lkp], BF16, tag="sq_k", name="sq_k"),
                "v": SQP.tile([128, NCT, Lkp], BF16, tag="sq_v", name="sq_v"),
            }

            psst_ctx = tc.tile_pool(name="psST", bufs=1, space="PSUM")
            PSST = psst_ctx.__enter__()
            NCH = {"q": NCQ, "k": NKT, "v": NKT}
            CH0 = {"q": 0, "k": 2 * NCQ, "v": 2 * NCQ + 2 * NKT}
            stW = 2 * NCQ + 4 * NKT
            st_ps = PSST.tile([128, stW], F32, tag="st", name="st")
            rq_ps = PSST.tile([128, 512], F32, tag="rqp", name="rqp")

            def warm_on(ap2d):
                # p-state anchor: 1-column matmul into stats col 0 (the real
                # chain's start=True reset later overwrites the garbage)
                nc.tensor.matmul(st_ps[0:ap2d.shape[1], 0:1], ap2d,
                                 ones_col[0:ap2d.shape[0], :],
                                 start=True, stop=True)

            # ------------------------------------------------------------
            # DMA issue (per-queue order = priority order)
            # ------------------------------------------------------------
            warm_on(ones_col)  # t~0.4us anchor, no DMA dependency
            # SP queue (k.h0 first: its transfer is the critical-path pole)
            nc.sync.dma_start(out=x_bf["k"][:, 0:4, 0:KH0], in_=kT_r[:, 0:4, 0:KH0])
            warm_on(x_bf["k"][:, 0, 0:1])
            nc.sync.dma_start(out=x_bf["k"][:, 4:8, 0:KH0], in_=kT_r[:, 4:8, 0:KH0])
            warm_on(x_bf["k"][:, 4, 0:1])
            nc.sync.dma_start(out=w_bf["k"], in_=wkT.rearrange("(j p) d -> p j d", p=128))
            warm_on(w_bf["k"][:, 0, 0:1])

            # ACT queue (q.h0 first)
            nc.scalar.dma_start(out=x_bf["q"][:, 0:4, 0:512], in_=qT_r[:, 0:4, 0:512])
            warm_on(x_bf["q"][:, 0, 0:1])
            nc.scalar.dma_start(out=x_bf["q"][:, 4:8, 0:512], in_=qT_r[:, 4:8, 0:512])
            warm_on(x_bf["q"][:, 4, 0:1])
            nc.scalar.dma_start(out=w_bf["q"], in_=wqT.rearrange("(j p) d -> p j d", p=128))
            warm_on(w_bf["q"][:, 0, 0:1])
            if not ln_identity:
                nc.scalar.dma_start(out=bwqk_sb, in_=bwqk_d[:, :])
                nc.scalar.dma_start(out=bwv_sb, in_=bwv_d[:, :])

            # Pool queue: v in quarters
            vq = _slices(Lkp, 256)
            for (o, s) in vq:
                nc.gpsimd.dma_start(out=x_bf["v"][:, :, o:o + s], in_=vT_r[:, :, o:o + s])

            vcol_sb = vb_sb[:, 0:NKT]
            bproj_sb = vb_sb[:, NKT:NKT + NDT]

            # ------------------------------------------------------------
            # stats helpers
            # ------------------------------------------------------------
            def emit_mu_chains(nm, chunks):
                xb = x_bf[nm]
                for c in chunks:
                    for j in range(NCT):
                        nc.tensor.matmul(st_ps[:, CH0[nm] + c:CH0[nm] + c + 1],
                                         xb[:, j, c * 128:(c + 1) * 128], ones_col,
                                         start=(j == 0), stop=(j == NCT - 1))

            def emit_msq_chains(nm, chunks):
                sq = sq_of[nm]
                base = CH0[nm] + NCH[nm]
                for c in chunks:
                    for j in range(NCT):
                        nc.tensor.matmul(st_ps[:, base + c:base + c + 1],
                                         sq[:, j, c * 128:(c + 1) * 128], ones_col,
                                         start=(j == 0), stop=(j == NCT - 1))

            def emit_sq(nm, j0, jn, o, s, eng):
                sq, xb = sq_of[nm], x_bf[nm]
                if eng == "act":
                    nc.scalar.activation(sq[:, j0:j0 + jn, o:o + s],
                                         xb[:, j0:j0 + jn, o:o + s],
                                         mybir.ActivationFunctionType.Square)
                elif eng == "pool":
                    nc.gpsimd.tensor_mul(sq[:, j0:j0 + jn, o:o + s],
                                         xb[:, j0:j0 + jn, o:o + s],
                                         xb[:, j0:j0 + jn, o:o + s])
                else:
                    nc.vector.tensor_mul(sq[:, j0:j0 + jn, o:o + s],
                                         xb[:, j0:j0 + jn, o:o + s],
                                         xb[:, j0:j0 + jn, o:o + s])

            stc = {nm: P.tile([128, 2 * NCH[nm]], F32, tag=f"stc_{nm}", name=f"stc_{nm}")
                   for nm in ("q", "k", "v")}
            sdc = {nm: P.tile([128, NCH[nm]], F32, tag=f"sdc_{nm}", name=f"sdc_{nm}")
                   for nm in ("q", "k", "v")}
            rsc = {nm: P.tile([128, NCH[nm]], F32, tag=f"rsc_{nm}", name=f"rsc_{nm}")
                   for nm in ("q", "k", "v")}
            rk_cols = P.tile([128, NKT], F32, tag="rk_cols", name="rk_cols")
            rq_sb = P.tile([128, 512], BF16, tag="rq_sb", name="rq_sb")

            def emit_stat_math(nm, c0, cn, cp="dve"):
                """Evict psum stat cols for chunks [c0, c0+cn) and derive
                rstd = rsqrt(var + eps) by a fixed-seed Newton iteration."""
                cpy = nc.scalar.copy if cp == "act" else nc.vector.tensor_copy
                mu = stc[nm][:, c0:c0 + cn]
                msq = stc[nm][:, NCH[nm] + c0:NCH[nm] + c0 + cn]
                cpy(mu, st_ps[:, CH0[nm] + c0:CH0[nm] + c0 + cn])
                cpy(msq, st_ps[:, CH0[nm] + NCH[nm] + c0:
                        CH0[nm] + NCH[nm] + c0 + cn])
                mu2 = SQ.tile([128, cn], F32, tag="mu2", name="mu2", bufs=2)
                if cp == "act":
                    nc.scalar.square(mu2, mu)
                else:
                    nc.vector.tensor_mul(mu2, mu, mu)
                var = sdc[nm][:, c0:c0 + cn]
                nc.vector.tensor_sub(var, msq, mu2)
                ne = nc.gpsimd if cp == "pool_nr" else nc.vector
                h = SQ.tile([128, cn], F32, tag="nrh", name="nrh", bufs=2)
                ne.tensor_scalar(h, var, -0.5, -0.5 * EPS,
                                 mybir.AluOpType.mult, mybir.AluOpType.add)
                y = rsc[nm][:, c0:c0 + cn]
                # seed at rsqrt of the expected token variance: ~2 for q/k
                # (x + pos doubles the variance), ~1 for v
                ne.memset(y, 1.0 if nm == "v" else 0.7071068)
                a = SQ.tile([128, cn], F32, tag="nra", name="nra", bufs=2)
                for _nr in range(3 if nm == "v" else 2):
                    ne.tensor_mul(a, y, y)
                    ne.tensor_mul(a, h, a)
                    ne.scalar_tensor_tensor(y, a, 1.5, y,
                                            mybir.AluOpType.add,
                                            mybir.AluOpType.mult)

            def emit_rq_rows(c0, cn, cp="dve"):
                # rq_ps[:, c*128+i] = rstd_q[i, c] for every partition:
                # matmul with a stride-0 (column-replicated) bf16 stationary
                # operand against the bf16 identity transposes AND broadcasts
                # in one cheap PE instruction per chunk
                rqb = SQ.tile([128, cn], BF16, tag="rqb", name="rqb", bufs=2)
                nc.vector.tensor_copy(rqb, rsc["q"][:, c0:c0 + cn])
                for c in range(cn):
                    col = rqb[:, c:c + 1]
                    rep = bass.AP(tensor=col.tensor, offset=col.offset,
                                  ap=[list(col.ap[0]), [0, 128]])
                    nc.tensor.matmul(rq_ps[:, c * 128:c * 128 + 128],
                                     rep, ident_sb, start=True, stop=True)
                nc.vector.tensor_copy(rq_sb[:, 0:cn * 128], rq_ps[:, 0:cn * 128])

            # ------------------------------------------------------------
            # projections
            # ------------------------------------------------------------
            q4 = P.tile([128, 2, LQ], BF16, tag="q4", name="q4")
            k4 = P.tile([128, 2, Lkp], BF16, tag="k4", name="k4")

            def emit_qk4(nm, dt, o0, s0, ev="dve"):
                dest = q4 if nm == "q" else k4
                for (o, s) in [(o0 + oo, ss) for (oo, ss) in _slices(s0, 512)]:
                    ps = PSA.tile([128, 512], F32, tag="main", name="main")
                    for j in range(NCT):
                        nc.tensor.matmul(ps[:, :s], w_bf[nm][:, j, dt * 128:(dt + 1) * 128],
                                         x_bf[nm][:, j, o:o + s], start=(j == 0),
                                         stop=(j == NCT - 1))
                    if nm == "q":
                        nc.vector.tensor_mul(dest[:, dt, o:o + s], ps[:, :s],
                                             rq_sb[:, (o % 512):(o % 512) + s])
                        if not ln_identity:
                            nc.vector.tensor_scalar_add(
                                dest[:, dt, o:o + s], dest[:, dt, o:o + s],
                                bwqk_sb[:, dt:dt + 1])
                    elif ln_identity:
                        if ev == "act":
                            nc.scalar.copy(dest[:, dt, o:o + s], ps[:, :s])
                        else:
                            nc.vector.tensor_copy(dest[:, dt, o:o + s], ps[:, :s])
                    else:
                        # fold rstd_k at the eviction (exp scale becomes the
                        # plain softmax SCALE constant), then add beta@W
                        nc.vector.tensor_mul(dest[:, dt, o:o + s], ps[:, :s],
                                             rq_sb[:, (o % 512):(o % 512) + s])
                        nc.vector.tensor_scalar_add(
                            dest[:, dt, o:o + s], dest[:, dt, o:o + s],
                            bwqk_sb[:, 2 + dt:3 + dt])

            v4a = P.tile([128, NKT, HPC * (CH + 1)], BF16, tag="v4a", name="v4a")
            v4a_h = v4a.rearrange("p k (h x) -> p k h x", h=HPC)

            def emit_v4(kts):
                for kt in kts:
                    ps = PSA.tile([128, 512], F32, tag="main", name="main")
                    for j in range(NCT):
                        nc.tensor.matmul(ps[:, :HC], x_bf["v"][:, j, kt * 128:(kt + 1) * 128],
                                         w_bf["v"][:, j, :], start=(j == 0), stop=False)
                    nc.tensor.matmul(ps[:, :HC], negmu_row["v"][:, kt * 128:(kt + 1) * 128],
                                     u_row["v"][:, :], start=False,
                                     stop=(bw_row["v"] is None))
                    if bw_row["v"] is not None:
                        nc.tensor.matmul(ps[:, :HC], sdrow["v"][:, kt * 128:(kt + 1) * 128],
                                         bw_row["v"][:, :], start=False, stop=True)
                    nc.vector.tensor_scalar_mul(
                        v4a_h[:, kt, :, 0:CH],
                        ps[:, 0:HC].rearrange("p (h x) -> p h x", h=HPC),
                        rsc["v"][:, kt:kt + 1])

            # ------------------------------------------------------------
            # attention pipeline
            # ------------------------------------------------------------
            groups = [(0, 512, 0), (0, 512, 1), (512, 512, 1), (512, 512, 0)]
            et_of = {}

            def emit_sexp(g, kts=None, alloc=True):
                (o, s, dt) = groups[g]
                if alloc:
                    et_of[g] = EP.tile([128, NKT, 2, 512], BF16, tag="E", name="E")
                et = et_of[g]
                for kt in (range(NKT) if kts is None else kts):
                    ps = PSS.tile([128, 2, 512], F32, tag="S", name="S")
                    for hh in range(2):
                        pb = 64 * hh
                        nc.tensor.matmul(ps[:, hh, :s],
                                         k4[pb:pb + CH, dt, kt * 128:(kt + 1) * 128],
                                         q4[pb:pb + CH, dt, o:o + s],
                                         start=True, stop=True)
                    nc.scalar.activation(et[:, kt, :, :s], ps[:, :, :s],
                                         mybir.ActivationFunctionType.Exp,
                                         scale=rk_cols[:, kt:kt + 1])

            o_sb = P.tile([128, HPC // 2, LQ], BF16, tag="o_sb", name="o_sb")
            ot_of = {}
            PSPV_box = [None]

            def emit_pv_unit(g, hh, qc):
                (o, s, dt) = groups[g]
                PSPV = PSPV_box[0]
                et = et_of[g]
                h = 2 * dt + hh
                qo = o + qc * 128
                ppv = PSPV.tile([128, CH + 1], F32, tag="pv", name="pv")
                for kt in range(NKT):
                    nc.tensor.matmul(ppv[:, :],
                                     et[:, kt, hh, qc * 128:(qc + 1) * 128],
                                     v4a[:, kt, h * (CH + 1):(h + 1) * (CH + 1)],
                                     start=(kt == 0), stop=(kt == NKT - 1))
                rdc = SQ.tile([128, 1], F32, tag="rdc", name="rdc", bufs=3)
                nc.vector.reciprocal_approx_fast(out=rdc, in_=ppv[:, CH:CH + 1])
                o_t = SQ.tile([128, CH], BF16, tag="o_t", name="o_t", bufs=17)
                nc.vector.tensor_scalar_mul(o_t, ppv[:, 0:CH], rdc)
                ot_of[(h, qo)] = o_t

            def emit_T_unit(o, qc):
                # transpose head pairs {t, t+2} of q-chunk qc into o_sb
                PSPV = PSPV_box[0]
                qo = o + qc * 128
                for t in range(2):
                    pst = PSPV.tile([128, 128], BF16, tag="pv", name="pv")
                    nc.tensor.transpose(pst[0:CH, :], ot_of.pop((t, qo)),
                                        ident_sb, tile_position=(0, 0))
                    nc.tensor.transpose(pst[CH:128, :], ot_of.pop((t + 2, qo)),
                                        ident_sb, tile_position=(0, CH))
                    nc.vector.tensor_copy(o_sb[:, t, qo:qo + 128], pst)

            def emit_proj_unit(dt2, o, s, evict, store):
                ps = PSA.tile([128, 512], F32, tag="main", name="main")
                for t in range(HPC // 2):
                    nc.tensor.matmul(ps[:, :s], wp_bf[:, t, dt2 * 128:(dt2 + 1) * 128],
                                     o_sb[:, t, o:o + s], start=(t == 0),
                                     stop=(t == HPC // 2 - 1))
                ot = SQ.tile([128, 512], BF16, tag="ot", name="ot", bufs=4)
                if not bproj_zero:
                    nc.vector.tensor_scalar_add(ot[:, :s], ps[:, :s],
                                                bproj_sb[:, dt2:dt2 + 1])
                elif evict == "act":
                    nc.scalar.activation(ot[:, :s], ps[:, :s],
                                         mybir.ActivationFunctionType.Copy)
                else:
                    nc.vector.tensor_copy(ot[:, :s], ps[:, :s])
                store.dma_start(out=out[dt2 * 128:(dt2 + 1) * 128, o:o + s],
                                in_=ot[:, :s])

            # ------------------------------------------------------------
            # ordered emission: pre-exp phase, then one unified PE stream
            # where score units (2 matmuls + 1 exp) interleave with filler
            # units (projections, v4, PV, transposes, out-proj) so the PE
            # queue never parks behind the exp-paced score tiles.
            # ------------------------------------------------------------
            emit_mu_chains("v", range(NKT))

            # q.h0: full squares + stat math on DVE (free early; shortest
            # latency to the first projections/scores)
            emit_sq("q", 0, 4, 0, 512, "dve")
            emit_sq("q", 4, 4, 0, 512, "dve")
            emit_mu_chains("q", range(4))
            emit_msq_chains("q", range(4))
            emit_stat_math("q", 0, 4)
            emit_rq_rows(0, 4)

            # k.h0: squares fully on ACT (its only pre-exp job); math on DVE
            emit_sq("k", 4, 4, 0, KH0, "act")
            emit_sq("k", 0, 4, 0, KH0, "act")
            emit_mu_chains("k", range(kh0c))
            emit_msq_chains("k", range(kh0c))
            emit_stat_math("k", 0, kh0c)
            nc.vector.tensor_scalar_mul(rk_cols[:, 0:kh0c], rsc["k"][:, 0:kh0c], SCALE)

            emit_qk4("q", 0, 0, 512)
            emit_qk4("k", 0, 0, KH0)

            # deferred second-half loads (gens issue behind the urgent ones)
            nc.sync.dma_start(out=x_bf["k"][:, 0:4, KH0:Lkp], in_=kT_r[:, 0:4, KH0:Lkp])
            nc.scalar.dma_start(out=x_bf["k"][:, 4:8, KH0:Lkp], in_=kT_r[:, 4:8, KH0:Lkp])
            nc.sync.dma_start(out=x_bf["q"][:, 0:4, 512:1024], in_=qT_r[:, 0:4, 512:1024])
            nc.scalar.dma_start(out=x_bf["q"][:, 4:8, 512:1024], in_=qT_r[:, 4:8, 512:1024])

            # k.h1 / v / q.h1 squares on Pool (non-critical latency)
            emit_sq("v", 0, NCT, vq[0][0], vq[0][1], "pool")
            emit_sq("v", 0, NCT, vq[1][0], vq[1][1], "pool")
            emit_sq("k", 0, 4, KH0, KH1, "pool")
            emit_sq("k", 4, 4, KH0, KH1, "pool")
            emit_sq("q", 4, 4, 512, 512, "pool")
            emit_sq("q", 0, 4, 512, 512, "pool")
            emit_sq("v", 0, NCT, vq[2][0], vq[2][1], "pool")
            emit_sq("v", 0, NCT, vq[3][0], vq[3][1], "pool")
            # deferred (non-urgent) loads: issue now, behind the urgent gens
            nc.sync.dma_start(out=w_bf["v"], in_=wvT.rearrange("(j p) d -> p j d", p=128))
            nc.scalar.dma_start(out=vb_sb, in_=vbpack[:, :])
            nc.sync.dma_start(out=wp_bf, in_=wp[:, :, :])

            # ---- filler units ----
            def f_kh1_stats():
                emit_mu_chains("k", range(kh0c, NKT))
                emit_msq_chains("k", range(kh0c, NKT))
                emit_stat_math("k", kh0c, NKT - kh0c)
                nc.vector.tensor_scalar_mul(rk_cols[:, kh0c:NKT],
                                            rsc["k"][:, kh0c:NKT], SCALE)

            def f_v_stats():
                emit_msq_chains("v", range(NKT))
                emit_stat_math("v", 0, NKT)
                for h in range(HPC):
                    nc.vector.tensor_copy(v4a_h[:, :, h, CH:CH + 1], vcol_sb[:, :, None])

            def f_qh1_stats():
                emit_mu_chains("q", range(4, NCQ))
                emit_msq_chains("q", range(4, NCQ))

            def f_qh1_math():
                emit_stat_math("q", 4, NCQ - 4)
                emit_rq_rows(4, NCQ - 4)

            def f_pools():
                sqp_ctx.__exit__(None, None, None)
                psst_ctx.__exit__(None, None, None)
                PSPV_box[0] = pspv_stack.enter_context(
                    tc.tile_pool(name="psPV", bufs=2, space="PSUM"))

            fillers = [
                (2.0, lambda: emit_qk4("q", 1, 0, 512)),
                (2.0, lambda: emit_qk4("k", 1, 0, KH0)),
                (0.3, f_kh1_stats),
                (2.0, lambda: emit_qk4("k", 0, KH0, KH1)),
                (2.0, lambda: emit_qk4("k", 1, KH0, KH1)),
                (0.1, f_qh1_stats),
                (0.4, f_qh1_math),
                (2.0, lambda: emit_qk4("q", 1, 512, 512)),
                (2.0, lambda: emit_qk4("q", 0, 512, 512)),
                (0.3, f_v_stats),
            ]
            for kt0 in range(NKT):
                fillers.append((1.0, lambda kt0=kt0: emit_v4([kt0])))
            fillers += [
                (0.0, f_pools),
            ]
            for hh in range(2):
                for qc in range(4):
                    fillers.append((0.25, lambda hh=hh, qc=qc: emit_pv_unit(0, hh, qc)))
            for qc in range(4):
                fillers.append((0.25, lambda qc=qc: emit_pv_unit(1, 0, qc)))
                fillers.append((0.25, lambda qc=qc: emit_pv_unit(1, 1, qc)))
                fillers.append((0.15, lambda qc=qc: emit_T_unit(0, qc)))
            for dt2 in range(NDT):
                fillers.append((0.5, lambda dt2=dt2: emit_proj_unit(dt2, 0, 512,
                                                                   "dve", nc.sync)))
            for hh in range(2):
                for qc in range(4):
                    fillers.append((0.25, lambda hh=hh, qc=qc: emit_pv_unit(2, hh, qc)))

            # ---- unified S stream: budget-paced filler pumping ----
            fi = [0]
            debt = [0.0]

            def pump(budget):
                debt[0] += budget
                while fi[0] < len(fillers) and debt[0] >= fillers[fi[0]][0]:
                    cost, fn = fillers[fi[0]]
                    fn()
                    debt[0] -= cost
                    fi[0] += 1

            def pump_through(idx):
                # force-emit fillers [0, idx) regardless of budget (ordering
                # requirement: later S units read tiles these fillers write)
                while fi[0] < idx:
                    fillers[fi[0]][1]()
                    fi[0] += 1

            # filler indices that must precede each score group / kt
            import os as _os
            _budget = float(_os.environ.get("PUMP_BUDGET", "1.0"))
            # g3's exp reuses E(g0)'s buffer: all 8 PV(g0) units (and the v4
            # chain they depend on) must drain before S(g3) starts
            need_g = {1: 5, 2: 9, 3: 10 + NKT}
            need_kh1 = 4
            for g in range(4):
                if g in need_g:
                    pump_through(need_g[g])
                emit_sexp(g, kts=[], alloc=True)
                for kt in range(NKT):
                    if g == 0 and kt == kh0c:
                        pump_through(need_kh1)
                    emit_sexp(g, kts=[kt], alloc=False)
                    # ~0.61us of exp-paced slack per S slot, minus overheads
                    pump(_budget)
            # flush remaining fillers behind the S stream
            debt[0] = 1e9
            pump(0.0)

            # ---- tail: PV(g3) per q-chunk, transposes, out-proj 256-wide ----
            for qc in range(4):
                emit_pv_unit(3, 0, qc)
                emit_pv_unit(3, 1, qc)
                emit_T_unit(512, qc)
                if qc == 1:
                    for dt2 in range(NDT):
                        emit_proj_unit(dt2, 512, 256,
                                       "act" if dt2 % 2 else "dve",
                                       nc.scalar if dt2 % 2 else nc.sync)
            for dt2 in range(NDT):
                emit_proj_unit(dt2, 768, 256,
                               "act" if dt2 % 2 else "dve",
                               nc.scalar if dt2 % 2 else nc.sync)

            pspv_stack.close()
            ep_ctx.__exit__(None, None, None)

    nc.compile()
    return nc


def prepare_in_maps(q, k, v, qpos, kpos, mask,
                    ln_q_w, ln_q_b, ln_k_w, ln_k_b, ln_v_w, ln_v_b,
                    w_q, w_k, w_v, w_proj, b_proj):
    import ml_dtypes
    bf = ml_dtypes.bfloat16
    f = np.float32
    q = np.asarray(q, f); k = np.asarray(k, f); v = np.asarray(v, f)
    qpos = np.asarray(qpos, f).reshape(B, LQ, C)
    kpos = np.asarray(kpos, f).reshape(B, LK, C)
    mask = np.asarray(mask)

    keeps = [np.flatnonzero(mask[b, 0, 0] == 0) for b in range(B)]
    Lkp = max(256, -(-max(len(kp) for kp in keeps) // 128) * 128)
    NKT = Lkp // 128

    def colmajor(vec, ntiles):
        return np.ascontiguousarray(vec.reshape(ntiles, 128).T.astype(f))

    gammas = {"q": np.asarray(ln_q_w, f), "k": np.asarray(ln_k_w, f),
              "v": np.asarray(ln_v_w, f)}
    betas = {"q": np.asarray(ln_q_b, f), "k": np.asarray(ln_k_b, f),
             "v": np.asarray(ln_v_b, f)}
    Ws = {"q": np.asarray(w_q, f), "k": np.asarray(w_k, f), "v": np.asarray(w_v, f)}
    ident = all(np.all(gammas[n] == 1.0) for n in "qkv") \
        and all(np.all(betas[n] == 0.0) for n in "qkv")

    in_maps = []
    for core in range(8):
        b, hg = core // 4, core % 4
        kp = keeps[b]
        nk = len(kp)
        hs = slice(hg * HC, (hg + 1) * HC)

        def padT(x2d):  # [n, C] -> [C, Lkp] bf16
            outp = np.zeros((C, Lkp), bf)
            outp[:, :x2d.shape[0]] = x2d.T.astype(bf)
            return np.ascontiguousarray(outp)

        # W'' = gamma-scaled W with the LayerNorm mean-centering folded in:
        # W''[c,d] = gamma[c]W[d,c] - u[d]/C with u = colsum(gamma*W), so
        # W''^T x = W'x - mu(x)*u exactly; bW = beta @ W
        wT_eff = {}
        bw_rows = np.zeros((3, HC), f)
        for i, nm in enumerate("qkv"):
            wt = (Ws[nm][hs, :] * gammas[nm][None, :]).T.astype(bf).astype(f)
            u = wt.sum(0)
            wT_eff[nm] = np.ascontiguousarray((wt - u[None, :] / C).astype(bf))
            bw_rows[i] = betas[nm].astype(f) @ Ws[nm][hs, :].T.astype(bf).astype(f)

        vcol_np = np.zeros(Lkp, f)
        vcol_np[:nk] = 1.0
        vb = np.concatenate(
            [colmajor(vcol_np, NKT),
             colmajor(np.asarray(b_proj, f) if hg == 0 else np.zeros(C, f), NDT)],
            axis=1)
        m = {
            "qT": np.ascontiguousarray((q[b] + qpos[b]).T.astype(bf)),
            "kT": padT((k[b] + kpos[b])[kp]),
            "vT": padT(v[b][kp]),
            "wqT": wT_eff["q"],
            "wkT": wT_eff["k"],
            "wvT": wT_eff["v"],
            # wp[64*(h//2)+p, h%2, d] = w_proj[d, hg*256 + 64h + p]:
            # heads 0/1 on the lower partition halves of slots 0/1, heads
            # 2/3 on the upper halves (matches the o_sb placement)
            "wp": np.ascontiguousarray(
                np.asarray(w_proj, f)[:, hs].T.reshape(2, HPC // 2, CH, C)
                .transpose(0, 2, 1, 3).reshape(128, HPC // 2, C).astype(bf)),
            "vbpack": np.ascontiguousarray(vb),
        }
        if not ident:
            # bias terms: per-partition columns for q/k (d on partitions, 2
            # dt tiles) and a partition-replicated tile for v (d on free dim)
            m["bwqk"] = np.ascontiguousarray(np.stack(
                [bw_rows[0].reshape(2, 128).T, bw_rows[1].reshape(2, 128).T],
                axis=1).reshape(128, 4).astype(f))
            m["bwv"] = np.ascontiguousarray(
                np.broadcast_to(bw_rows[2][None, :], (128, HC)).astype(bf))
        in_maps.append(m)
    return in_maps, Lkp, ident


def kernel(**inputs):
    global LAST_EXEC_NS, LAST_RESULTS
    f = np.float32
    in_maps, Lkp, ident = prepare_in_maps(**inputs)
    bz = bool(np.all(np.asarray(inputs["b_proj"]) == 0.0))
    key = (Lkp, ident, bz)
    nc = _NC_CACHE.get(key)
    if nc is None:
        nc = build_nc(Lkp, ln_identity=ident, bproj_zero=bz)
        _NC_CACHE[key] = nc
    trace = os.environ.get("KERNEL_TRACE", "0") == "1"
    res = run_bass_kernel_spmd(nc, in_maps, core_ids=list(range(8)), trace=trace)
    LAST_EXEC_NS = res.exec_time_ns
    LAST_RESULTS = res

    out_full = np.zeros((B, LQ, C), f)
    for b in range(B):
        acc = np.zeros((C, LQ), f)
        for hg in range(4):
            acc += res.results[b * 4 + hg]["out"].astype(f)
        out_full[b] = acc.T
    return out_full
